# revision 27
# baseline (speedup 1.0000x reference)
"""Trainium2 Bass kernel for the 2-layer GAT model (top-10 attention, 4 heads).

Strategy (8 NeuronCores, SPMD):
- Nodes sharded into 8 contiguous ranges of 6250 (dst ranges == GEMM shards).
  Within each core, dst nodes are degree-sorted (host permutation) into 49
  tiles of 128 with a per-tile slot count D[t] shared across cores.
- Two replicated tables per layer: messages xl [NT, 256] f16 (512B rows) and
  attention logits asd [NT, 64] f32 (256B rows; only 8 cols used). The logit
  path stays f32 end-to-end (f16/bf16 logits reorder the top-10 selection
  vs the reference and blow the error budget); messages are f16 (halves
  gather + AllGather traffic vs f32).
- GEMM per shard: message matmuls in f16, att-projection matmuls in f32
  (folded as 8 extra f32 columns); AllGather both tables (27MB + 1.6MB).
  Pad slots point at a poison row in asd (a_s = -1e30) so no explicit pad
  masking is needed; its message row is zero.
- Edge phase: per group of tiles (sum D <= 64) ONE index tile feeds TWO
  dma_gathers (256B logit rows + 512B message rows); alpha = a_s + a_d,
  leaky via fused scalar_tensor_tensor, top-10 threshold via DVE
  max8/match_replace/max8, exp on the scalar engine (f16 out), per-slot f16
  identity-matmul accumulation into PSUM, per-head normalize on the scalar
  engine. b1 folds into GEMM2 (ones-row matmul of b1 @ W2_ext); b2 into the
  MLP bias. Head-mean + 2-layer MLP fused per tile.
"""
import numpy as np

N = 50000
E = 800000
F_IN = 256
H, C = 4, 64
HC = H * C
K_TOP = 10
NEG_SLOPE = 0.2
N_CORES = 8
SH = N // N_CORES            # 6250 real rows per core
TILES = (SH + 127) // 128    # 49
ROWS = TILES * 128           # 6272 padded rows per core
NT = N_CORES * ROWS          # 50176 global table rows
BASE = 32768                 # signed-int16 gather base row
PADROW = NT - 1              # poison row (msg = 0, a_s = -1e30)
HID, OUT_F = 128, 16
GROUP_CAP = 32               # max sum of D over one gather group


def _wrap_idx(vals: np.ndarray) -> np.ndarray:
    """int16 index list -> [128, ceil(len/16)] wrapped+replicated tile."""
    ni = len(vals)
    w = -(-ni // 16)
    arr = np.full(w * 16, PADROW - BASE, np.int16)
    arr[:ni] = vals
    return np.tile(arr.reshape(w, 16).T, (8, 1))


def _prep(x, W1, att_s1, att_d1, W2, att_s2, att_d2, Wl1, Wl2, edge_index,
          b1, b2, bl1):
    """Host preprocessing: sharding, degree-sorted tiles, gather index tables."""
    src = np.asarray(edge_index[0], np.int64)
    dst = np.asarray(edge_index[1], np.int64)

    deg = np.bincount(dst, minlength=N)
    loc = np.empty(N, np.int64)
    node_of = np.full((N_CORES, ROWS), -1, np.int64)
    for c in range(N_CORES):
        nodes = np.arange(c * SH, (c + 1) * SH)
        order = np.argsort(-deg[nodes], kind="stable")
        loc[nodes[order]] = np.arange(SH)
        node_of[c, :SH] = nodes[order]
    rowid = (np.arange(N) // SH) * ROWS + loc  # node -> global table row

    degl = np.zeros((N_CORES, ROWS), np.int64)
    for c in range(N_CORES):
        degl[c, :SH] = deg[node_of[c, :SH]]
    tile_max = degl.reshape(N_CORES, TILES, 128).max(axis=(0, 2))
    D = np.maximum(8, ((tile_max + 3) // 4) * 4).astype(np.int64)
    assert D.max() <= 64, f"degree too high: {D.max()}"

    groups = []
    cur, tot = [], 0
    for t in range(TILES):
        if cur and tot + int(D[t]) > GROUP_CAP:
            groups.append(cur)
            cur, tot = [], 0
        cur.append(t)
        tot += int(D[t])
    groups.append(cur)

    # CSR of edges by (core, local dst row)
    e_loc = (dst // SH) * ROWS + loc[dst]
    order_e = np.argsort(e_loc, kind="stable")
    src_s = src[order_e]
    e_loc_s = e_loc[order_e]
    starts = np.searchsorted(e_loc_s, np.arange(N_CORES * ROWS))
    ends = np.searchsorted(e_loc_s, np.arange(N_CORES * ROWS) + 1)

    PAD16 = np.int16(PADROW - BASE)
    idx_parts = [[] for _ in range(N_CORES)]
    for c in range(N_CORES):
        tile_s16 = []
        for t in range(TILES):
            Dt = int(D[t])
            slot = np.full((128, Dt), PADROW, np.int64)
            for d in range(128):
                r = c * ROWS + t * 128 + d
                s, e = starts[r], ends[r]
                if e > s:
                    slot[d, : e - s] = rowid[src_s[s:e]]
            tile_s16.append((slot - BASE).astype(np.int16).T.reshape(-1))
        for tg in groups:
            vals = np.concatenate([tile_s16[t] for t in tg] + [[PAD16]])
            idx_parts[c].append(_wrap_idx(vals))
    idx = np.stack([np.concatenate(p, axis=1) for p in idx_parts])

    x_shardT = np.zeros((N_CORES, F_IN, ROWS), np.float32)
    xf = np.asarray(x, np.float32)
    for c in range(N_CORES):
        x_shardT[c, :, :SH] = xf[node_of[c, :SH]].T

    def att_fold(WT, att_s, att_d):
        Vs = np.stack([WT[:, h * C: (h + 1) * C] @ np.asarray(att_s)[0, h]
                       for h in range(H)], axis=1)
        Vd = np.stack([WT[:, h * C: (h + 1) * C] @ np.asarray(att_d)[0, h]
                       for h in range(H)], axis=1)
        return np.hstack([Vs, Vd]).astype(np.float32)

    W1T = np.asarray(W1).T.astype(np.float32)
    W2T = np.asarray(W2).T.astype(np.float32)
    V1 = att_fold(W1T, att_s1, att_d1)            # [F_IN, 8] f32
    V2 = att_fold(W2T, att_s2, att_d2)            # [HC, 8] f32
    b1f = np.asarray(b1, np.float32)
    b1w2m = (b1f @ W2T)[None, :]                  # [1, 256]
    b1w2a = (b1f @ V2)[None, :]                   # [1, 8]
    bl1p = (np.asarray(bl1, np.float32)
            + np.asarray(Wl1, np.float32) @ np.asarray(b2, np.float32))

    meta = dict(D=[int(d) for d in D], groups=groups)
    consts = dict(
        W1m=W1T.astype(np.float16), W2m=W2T.astype(np.float16),
        V1=V1, V2=V2,
        b1w2m=b1w2m.astype(np.float16), b1w2a=b1w2a,
        Wl1T=np.asarray(Wl1).T.astype(np.float32).copy(),
        Wl2T=np.asarray(Wl2).T.astype(np.float32).copy(),
        bl1_col=np.ascontiguousarray(bl1p[:, None]),
    )
    per_core = dict(x_shardT=x_shardT, idx=idx)
    return meta, consts, per_core, node_of


def build_gnn(meta, repeat=1, stage=6):
    """stage: 1=gemm1, 2=+ag1, 3=+edge1, 4=+gemm2, 5=+ag2, 6=full."""
    from concourse import bass, bacc, mybir
    import concourse.tile as tile
    from concourse.masks import make_identity

    D = meta["D"]
    groups = meta["groups"]
    WAS = sum(-(-(128 * sum(D[t] for t in tg) + 1) // 16) for tg in groups)

    f32 = mybir.dt.float32
    f16 = mybir.dt.float16
    i16 = mybir.dt.int16
    nc = bacc.Bacc(None, target_bir_lowering=False, num_devices=N_CORES,
                   num_swdge_queues=4)

    xT_in = nc.dram_tensor("x_shardT", [F_IN, ROWS], f32, kind="ExternalInput")
    w1_in = nc.dram_tensor("W1m", [F_IN, HC], f16, kind="ExternalInput")
    w2_in = nc.dram_tensor("W2m", [HC, HC], f16, kind="ExternalInput")
    v1_in = nc.dram_tensor("V1", [F_IN, 8], f32, kind="ExternalInput")
    v2_in = nc.dram_tensor("V2", [HC, 8], f32, kind="ExternalInput")
    bwm_in = nc.dram_tensor("b1w2m", [1, HC], f16, kind="ExternalInput")
    bwa_in = nc.dram_tensor("b1w2a", [1, 8], f32, kind="ExternalInput")
    wl1_in = nc.dram_tensor("Wl1T", [C, HID], f32, kind="ExternalInput")
    wl2_in = nc.dram_tensor("Wl2T", [HID, OUT_F], f32, kind="ExternalInput")
    bl1_in = nc.dram_tensor("bl1_col", [HID, 1], f32, kind="ExternalInput")
    bl2_in = nc.dram_tensor("bl2_rep", [128, OUT_F], f32, kind="ExternalInput")
    ia_in = nc.dram_tensor("idx", [128, WAS], i16, kind="ExternalInput")

    out_dram = nc.dram_tensor("out", [ROWS, OUT_F], f32, kind="ExternalOutput")

    xl_sh = [nc.dram_tensor(f"xl{l}_shard", [ROWS, HC], f16) for l in (1, 2)]
    xl_fu = [nc.dram_tensor(f"xl{l}_full", [NT, HC], f16, addr_space="Shared")
             for l in (1, 2)]
    asd_lo = [nc.dram_tensor(f"asd{l}_local", [ROWS, 8], f32) for l in (1, 2)]
    asd_fu = [nc.dram_tensor(f"asd{l}_full", [NT, 8], f32, addr_space="Shared")
              for l in (1, 2)]
    asd_pad = [nc.dram_tensor(f"asd{l}_pad", [NT, 64], f32) for l in (1, 2)]
    out1_dram = nc.dram_tensor("out1_dram", [ROWS, HC], f32)

    CPY = mybir.ActivationFunctionType.Copy
    EXP = mybir.ActivationFunctionType.Exp
    RELU = mybir.ActivationFunctionType.Relu
    MAXO = mybir.AluOpType.max
    ADD = mybir.AluOpType.add
    MUL = mybir.AluOpType.mult
    SUB = mybir.AluOpType.subtract
    LT = mybir.AluOpType.is_lt
    GT = mybir.AluOpType.is_gt
    X = mybir.AxisListType.X

    with tile.TileContext(nc) as tc:
        with (
            tc.tile_pool(name="const", bufs=1) as cpool,
            tc.tile_pool(name="gemm", bufs=4) as gpool,
            tc.tile_pool(name="gpsum", bufs=2, space="PSUM") as gpsum,
            tc.tile_pool(name="idxp", bufs=3) as ipool,
            tc.tile_pool(name="edge", bufs=4) as epool,
            tc.tile_pool(name="small", bufs=8) as spool,
            tc.tile_pool(name="agg", bufs=2, space="PSUM") as apsum,
            tc.tile_pool(name="mlpp", bufs=1, space="PSUM") as mpsum,
        ):
            # ---- constants ----
            ident = cpool.tile([128, 128], f32)
            make_identity(nc, ident[:])
            identh = cpool.tile([128, 128], f16)
            nc.vector.tensor_copy(identh[:], ident[:])
            ones16 = cpool.tile([1, 128], f16)
            nc.vector.memset(ones16[:], 1.0)
            ones32 = cpool.tile([1, 128], f32)
            nc.vector.memset(ones32[:], 1.0)
            poison = cpool.tile([1, 4], f32)
            nc.vector.memset(poison[:], -1e30)
            w1_sb = cpool.tile([128, 2, HC], f16)
            nc.sync.dma_start(out=w1_sb[:, 0], in_=w1_in[0:128])
            nc.sync.dma_start(out=w1_sb[:, 1], in_=w1_in[128:256])
            w2_sb = cpool.tile([128, 2, HC], f16)
            nc.sync.dma_start(out=w2_sb[:, 0], in_=w2_in[0:128])
            nc.sync.dma_start(out=w2_sb[:, 1], in_=w2_in[128:256])
            v1_sb = cpool.tile([128, 2, 8], f32)
            nc.sync.dma_start(out=v1_sb[:, 0], in_=v1_in[0:128])
            nc.sync.dma_start(out=v1_sb[:, 1], in_=v1_in[128:256])
            v2_sb = cpool.tile([128, 2, 8], f32)
            nc.sync.dma_start(out=v2_sb[:, 0], in_=v2_in[0:128])
            nc.sync.dma_start(out=v2_sb[:, 1], in_=v2_in[128:256])
            bwm_sb = cpool.tile([1, HC], f16)
            nc.sync.dma_start(out=bwm_sb[:], in_=bwm_in[:])
            bwa_sb = cpool.tile([1, 8], f32)
            nc.sync.dma_start(out=bwa_sb[:], in_=bwa_in[:])
            wl1_sb = cpool.tile([C, HID], f32)
            nc.sync.dma_start(out=wl1_sb[:], in_=wl1_in[:])
            wl2_sb = cpool.tile([HID, OUT_F], f32)
            nc.sync.dma_start(out=wl2_sb[:], in_=wl2_in[:])
            bl1_sb = cpool.tile([HID, 1], f32)
            nc.sync.dma_start(out=bl1_sb[:], in_=bl1_in[:])
            bl2_sb = cpool.tile([128, OUT_F], f32)
            nc.sync.dma_start(out=bl2_sb[:], in_=bl2_in[:])
            # all gather-index tables preloaded once (reused by both layers):
            # keeps per-group idx loads out of the in-order SP DMA queue,
            # which otherwise serializes gathers behind compute-dependent
            # output writes
            ia_sb = cpool.tile([128, WAS], i16)
            nc.sync.dma_start(out=ia_sb[:], in_=ia_in[:])

            qrr = [0]

            def gemm_phase(l, w_sb, v_sb):
                li = l - 1
                for t in range(TILES):
                    rows = slice(t * 128, (t + 1) * 128)
                    xT32 = gpool.tile([128, 2, 128], f32, tag="g_T32")
                    xT16 = gpool.tile([128, 2, 128], f16, tag="g_T16")
                    if l == 1:
                        nc.sync.dma_start(out=xT32[:, 0], in_=xT_in[0:128, rows])
                        nc.sync.dma_start(out=xT32[:, 1], in_=xT_in[128:256, rows])
                        for k in range(2):
                            nc.scalar.activation(xT16[:, k], xT32[:, k], CPY)
                    else:
                        o1t = gpool.tile([128, HC], f32, tag="g_in")
                        nc.sync.dma_start(out=o1t[:], in_=out1_dram[rows])
                        for k in range(2):
                            pst = gpsum.tile([128, 128], f32, tag="g_tp")
                            nc.tensor.transpose(
                                pst[:], o1t[:, k * 128: (k + 1) * 128], ident[:])
                            nc.scalar.activation(xT32[:, k], pst[:], CPY)
                            nc.scalar.activation(xT16[:, k], pst[:], CPY)
                    ps = gpsum.tile([128, HC + 8], f32, tag="g_mm")
                    nc.tensor.matmul(ps[:, 0:HC], xT16[:, 0], w_sb[:, 0],
                                     start=True, stop=False)
                    nc.tensor.matmul(ps[:, 0:HC], xT16[:, 1], w_sb[:, 1],
                                     start=False, stop=(l == 1))
                    if l == 2:
                        nc.tensor.matmul(ps[:, 0:HC], ones16[:], bwm_sb[:],
                                         start=False, stop=True)
                    nc.tensor.matmul(ps[:, HC: HC + 8], xT32[:, 0], v_sb[:, 0],
                                     start=True, stop=False)
                    nc.tensor.matmul(ps[:, HC: HC + 8], xT32[:, 1], v_sb[:, 1],
                                     start=False, stop=(l == 1))
                    if l == 2:
                        nc.tensor.matmul(ps[:, HC: HC + 8], ones32[:], bwa_sb[:],
                                         start=False, stop=True)
                    ogm = gpool.tile([128, HC], f16, tag="g_om")
                    nc.scalar.activation(ogm[:], ps[:, 0:HC], CPY)
                    oga = gpool.tile([128, 8], f32, tag="g_oa")
                    nc.vector.tensor_copy(oga[:], ps[:, HC: HC + 8])
                    nc.sync.dma_start(out=xl_sh[li][rows], in_=ogm[:])
                    nc.sync.dma_start(out=asd_lo[li][rows], in_=oga[:])

            def allgather(l):
                li = l - 1
                nc.gpsimd.collective_compute(
                    "AllGather", mybir.AluOpType.bypass,
                    replica_groups=[list(range(N_CORES))],
                    ins=[asd_lo[li].ap().opt()], outs=[asd_fu[li].ap().opt()],
                )
                nc.sync.dma_start(out=asd_pad[li][:, 0:8], in_=asd_fu[li][:])
                nc.sync.dma_start(
                    out=asd_pad[li][PADROW: PADROW + 1, 0:4], in_=poison[:])
                nc.gpsimd.collective_compute(
                    "AllGather", mybir.AluOpType.bypass,
                    replica_groups=[list(range(N_CORES))],
                    ins=[xl_sh[li].ap().opt()], outs=[xl_fu[li].ap().opt()],
                )

            def do_norm(l, rows, ps, inv):
                # normalize + (layer 2) head-mean + MLP; lags one tile so the
                # vector engine is not blocked waiting on this tile's matmuls
                if l == 1:
                    o1 = epool.tile([128, HC], f32, tag="o1")
                    for h in range(H):
                        cs = slice(h * C, (h + 1) * C)
                        nc.scalar.activation(o1[:, cs], ps[:, cs], CPY,
                                             scale=inv[:, h: h + 1])
                    nc.sync.dma_start(out=out1_dram[rows], in_=o1[:])
                else:
                    o2 = spool.tile([128, C], f32, tag="o2")
                    nc.vector.tensor_scalar(
                        out=o2[:], in0=ps[:, 0:C],
                        scalar1=inv[:, 0:1], scalar2=None, op0=MUL)
                    for h in range(1, H):
                        cs = slice(h * C, (h + 1) * C)
                        nc.vector.scalar_tensor_tensor(
                            out=o2[:], in0=ps[:, cs],
                            scalar=inv[:, h: h + 1], in1=o2[:],
                            op0=MUL, op1=ADD)
                    psT = mpsum.tile([C, 128], f32, tag="m_th")
                    nc.tensor.transpose(psT[:], o2[:], ident[:])
                    o2T = spool.tile([C, 128], f32, tag="o2T")
                    nc.scalar.activation(o2T[:], psT[:], CPY)
                    psh = mpsum.tile([HID, 128], f32, tag="m_th")
                    nc.tensor.matmul(psh[:], wl1_sb[:], o2T[:],
                                     start=True, stop=True)
                    rh = spool.tile([HID, 128], f32, tag="rh")
                    nc.scalar.activation(rh[:], psh[:], RELU, bias=bl1_sb[:])
                    pso = mpsum.tile([OUT_F, 128], f32, tag="m_of")
                    nc.tensor.matmul(pso[:], wl2_sb[:], rh[:],
                                     start=True, stop=True)
                    po = spool.tile([OUT_F, 128], f32, tag="po")
                    nc.vector.tensor_copy(po[:], pso[:])
                    psf = mpsum.tile([128, OUT_F], f32, tag="m_of")
                    nc.tensor.transpose(psf[:], po[:], ident[:OUT_F, :OUT_F])
                    of = spool.tile([128, OUT_F], f32, tag="of")
                    nc.vector.tensor_tensor(out=of[:], in0=psf[:],
                                            in1=bl2_sb[:], op=ADD)
                    nc.sync.dma_start(out=out_dram[rows], in_=of[:])

            pending = [None]

            def edge_phase(l):
                li = l - 1
                adr = (asd_lo[li].ap()
                       .rearrange("(t d) c -> d t c", d=128)[:, :, 4:8])
                ad_all = cpool.tile([128, TILES, 4], f32, tag=f"ad{l}")
                nc.sync.dma_start(out=ad_all[:], in_=adr)

                oas = 0
                for tg in groups:
                    SG = sum(D[t] for t in tg)
                    wg = -(-(128 * SG + 1) // 16)
                    ixt = ia_sb[:, oas: oas + wg]
                    oas += wg
                    asg = epool.tile([128, SG + 1, 64], f32, tag="asg")
                    nc.gpsimd.dma_gather(
                        out_ap=asg[:], in_ap=asd_pad[li][BASE:, :],
                        idxs_ap=ixt, num_idxs=128 * SG + 1,
                        num_idxs_reg=128 * SG + 1, elem_size=64,
                        single_packet=False, queue_num=qrr[0] % 4,
                    )
                    qrr[0] += 1
                    xg = epool.tile([128, SG + 1, HC], f16, tag="xg")
                    nc.gpsimd.dma_gather(
                        out_ap=xg[:], in_ap=xl_fu[li][BASE:, :], idxs_ap=ixt,
                        num_idxs=128 * SG + 1, num_idxs_reg=128 * SG + 1,
                        elem_size=HC, single_packet=False,
                        queue_num=qrr[0] % 4,
                    )
                    qrr[0] += 1
                    off = 0
                    for t in tg:
                        Dt = D[t]
                        rows = slice(t * 128, (t + 1) * 128)
                        msl = xg[:, off: off + Dt]
                        alpha = spool.tile([128, H, Dt], f32, tag="alpha")
                        nc.vector.tensor_tensor(
                            out=alpha[:],
                            in0=asg[:, off: off + Dt, 0:4].transpose([0, 2, 1]),
                            in1=ad_all[:, t].unsqueeze(2)
                                .broadcast_to([128, H, Dt]),
                            op=ADD,
                        )
                        nc.vector.scalar_tensor_tensor(
                            out=alpha[:], in0=alpha[:], scalar=NEG_SLOPE,
                            in1=alpha[:], op0=MUL, op1=MAXO,
                        )
                        m8a = spool.tile([128, H, 8], f32, tag="m8a")
                        exm = spool.tile([128, H, Dt], f32, tag="exm")
                        if K_TOP < Dt <= 17:
                            # rank-10-largest == ascending rank Dt-10 (<8):
                            # ONE max8 on -alpha per head replaces
                            # max8 + match_replace + max8
                            nal = spool.tile([128, H, Dt], f32, tag="nal")
                            nc.vector.tensor_scalar_mul(nal[:], alpha[:], -1.0)
                            nc.vector.reduce_max(out=m8a[:, :, 0], in_=alpha[:],
                                                 axis=X)
                            m8n = spool.tile([128, H, 8], f32, tag="m8n")
                            for h in range(H):
                                nc.vector.max(out=m8n[:, h], in_=nal[:, h])
                            nc.vector.tensor_tensor(
                                out=exm[:], in0=alpha[:],
                                in1=m8a[:, :, 0:1].broadcast_to([128, H, Dt]),
                                op=SUB)
                            mlt = spool.tile([128, H, Dt], f32, tag="mlt")
                            ks = Dt - K_TOP
                            nc.vector.tensor_tensor(
                                out=mlt[:], in0=nal[:],
                                in1=m8n[:, :, ks: ks + 1]
                                    .broadcast_to([128, H, Dt]),
                                op=GT)
                            nc.vector.scalar_tensor_tensor(
                                out=exm[:], in0=mlt[:], scalar=-1e30,
                                in1=exm[:], op0=MUL, op1=ADD)
                        elif Dt > K_TOP:
                            m8b = spool.tile([128, H, 8], f32, tag="m8b")
                            wk = spool.tile([128, Dt], f32, tag="wk")
                            for h in range(H):
                                nc.vector.max(out=m8a[:, h], in_=alpha[:, h])
                                nc.vector.match_replace(
                                    out=wk[:], in_to_replace=m8a[:, h],
                                    in_values=alpha[:, h], imm_value=-3e30)
                                nc.vector.max(out=m8b[:, h], in_=wk[:])
                            nc.vector.tensor_tensor(
                                out=exm[:], in0=alpha[:],
                                in1=m8a[:, :, 0:1].broadcast_to([128, H, Dt]),
                                op=SUB)
                            mlt = spool.tile([128, H, Dt], f32, tag="mlt")
                            nc.vector.tensor_tensor(
                                out=mlt[:], in0=alpha[:],
                                in1=m8b[:, :, 1:2].broadcast_to([128, H, Dt]),
                                op=LT)
                            nc.vector.scalar_tensor_tensor(
                                out=exm[:], in0=mlt[:], scalar=-1e30,
                                in1=exm[:], op0=MUL, op1=ADD)
                        else:
                            nc.vector.reduce_max(out=m8a[:, :, 0], in_=alpha[:],
                                                 axis=X)
                            nc.vector.tensor_scalar_max(
                                m8a[:, :, 0], m8a[:, :, 0], -1e29)
                            nc.vector.tensor_tensor(
                                out=exm[:], in0=alpha[:],
                                in1=m8a[:, :, 0:1].broadcast_to([128, H, Dt]),
                                op=SUB)
                        exb = spool.tile([128, H, Dt], f16, tag="exb")
                        nc.scalar.activation(exb[:], exm[:], EXP)
                        den = spool.tile([128, H], f32, tag="den")
                        nc.vector.reduce_sum(out=den[:], in_=exb[:], axis=X)
                        inv = spool.tile([128, H], f32, tag="inv")
                        if l == 1:
                            nc.vector.tensor_scalar_max(den[:], den[:], 1e-20)
                        else:
                            nc.vector.tensor_scalar(
                                out=den[:], in0=den[:], scalar1=1e-20,
                                scalar2=float(H), op0=MAXO, op1=MUL)
                        nc.vector.reciprocal(inv[:], den[:])
                        nc.vector.tensor_tensor(
                            out=msl[:].rearrange("p j (h c) -> p j h c", h=H),
                            in0=msl[:].rearrange("p j (h c) -> p j h c", h=H),
                            in1=exb[:].transpose([0, 2, 1]).unsqueeze(3)
                                .broadcast_to([128, Dt, H, C]),
                            op=MUL,
                        )
                        ps = apsum.tile([128, HC], f32, tag="agg")
                        for j in range(Dt):
                            nc.tensor.matmul(
                                ps[:], identh[:], xg[:, off + j],
                                start=(j == 0), stop=(j == Dt - 1))
                        prev = pending[0]
                        pending[0] = (rows, ps, inv)
                        if prev is not None:
                            do_norm(l, *prev)
                        off += Dt
                if pending[0] is not None:
                    do_norm(l, *pending[0])
                    pending[0] = None

            def edge_gather_only(l):
                # stage-7 probe: run the full gather stream of one edge phase
                # with minimal consumption, to isolate DMA cost
                li = l - 1
                cons = cpool.tile([128, 8], f32, tag="cons")
                nc.vector.memset(cons[:], 0.0)
                oas = 0
                for tg in groups:
                    SG = sum(D[t] for t in tg)
                    wg = -(-(128 * SG + 1) // 16)
                    ixt = ia_sb[:, oas: oas + wg]
                    oas += wg
                    asg = epool.tile([128, SG + 1, 64], f32, tag="asg")
                    nc.gpsimd.dma_gather(
                        out_ap=asg[:], in_ap=asd_pad[li][BASE:, :],
                        idxs_ap=ixt, num_idxs=128 * SG + 1,
                        num_idxs_reg=128 * SG + 1, elem_size=64,
                        single_packet=False, queue_num=qrr[0] % 4,
                    )
                    qrr[0] += 1
                    xg = epool.tile([128, SG + 1, HC], f16, tag="xg")
                    nc.gpsimd.dma_gather(
                        out_ap=xg[:], in_ap=xl_fu[li][BASE:, :], idxs_ap=ixt,
                        num_idxs=128 * SG + 1, num_idxs_reg=128 * SG + 1,
                        elem_size=HC, single_packet=False,
                        queue_num=qrr[0] % 4,
                    )
                    qrr[0] += 1
                    xc = spool.tile([128, 8], f32, tag="xc")
                    nc.vector.tensor_copy(xc[:], xg[:, 0, 0:8])
                    nc.vector.tensor_tensor(out=cons[:], in0=cons[:],
                                            in1=xc[:], op=ADD)
                    nc.vector.tensor_tensor(out=cons[:], in0=cons[:],
                                            in1=asg[:, 0, 0:8], op=ADD)
                nc.sync.dma_start(out=out_dram[0:128, 0:8], in_=cons[:])

            def debug_touch(src_dram, is16):
                # read back a strip so partial stages aren't dead code
                for t in range(TILES):
                    rows = slice(t * 128, (t + 1) * 128)
                    dt_ = spool.tile([128, OUT_F], f16 if is16 else f32,
                                     tag="dbg")
                    nc.sync.dma_start(out=dt_[:], in_=src_dram[rows, 0:OUT_F])
                    df = spool.tile([128, OUT_F], f32, tag="dbgf")
                    nc.vector.tensor_copy(df[:], dt_[:])
                    nc.sync.dma_start(out=out_dram[rows], in_=df[:])

            for _rep in range(repeat):
                if stage == 7:
                    gemm_phase(1, w1_sb, v1_sb)
                    allgather(1)
                    edge_gather_only(1)
                    continue
                gemm_phase(1, w1_sb, v1_sb)
                if stage >= 2:
                    allgather(1)
                if stage >= 3:
                    edge_phase(1)
                if stage >= 4:
                    gemm_phase(2, w2_sb, v2_sb)
                if stage >= 5:
                    allgather(2)
                if stage >= 6:
                    edge_phase(2)
            if stage < 6:
                if stage == 1:
                    debug_touch(xl_sh[0], True)
                elif stage == 2:
                    debug_touch(xl_fu[0], True)
                    debug_touch(asd_pad[0], False)
                elif stage == 3:
                    debug_touch(out1_dram, False)
                elif stage == 4:
                    debug_touch(xl_sh[1], True)
                elif stage == 5:
                    debug_touch(xl_fu[1], True)
                    debug_touch(asd_pad[1], False)

    nc.compile()
    return nc


def _make_in_maps(consts, per_core, bl2):
    bl2 = np.asarray(bl2, np.float32)
    shared = dict(
        W1m=consts["W1m"], W2m=consts["W2m"], V1=consts["V1"], V2=consts["V2"],
        b1w2m=consts["b1w2m"], b1w2a=consts["b1w2a"],
        Wl1T=consts["Wl1T"], Wl2T=consts["Wl2T"],
        bl1_col=consts["bl1_col"],
        bl2_rep=np.tile(bl2[None, :], (128, 1)),
    )
    return [
        dict(
            shared,
            x_shardT=np.ascontiguousarray(per_core["x_shardT"][c]),
            idx=np.ascontiguousarray(per_core["idx"][c]),
        )
        for c in range(N_CORES)
    ]


def _assemble(results, node_of):
    out = np.empty((N, OUT_F), np.float32)
    for c in range(N_CORES):
        out[node_of[c, :SH]] = results[c]["out"][:SH]
    return out


def kernel(x, W1, att_s1, att_d1, b1, W2, att_s2, att_d2, b2,
           Wl1, bl1, Wl2, bl2, edge_index):
    from concourse.bass_utils import run_bass_kernel_spmd

    meta, consts, per_core, node_of = _prep(
        x, W1, att_s1, att_d1, W2, att_s2, att_d2, Wl1, Wl2, edge_index,
        b1, b2, bl1,
    )
    nc = build_gnn(meta)
    in_maps = _make_in_maps(consts, per_core, bl2)
    res = run_bass_kernel_spmd(nc, in_maps, core_ids=list(range(N_CORES)))
    return _assemble(res.results, node_of)


# revision 31
# speedup vs baseline: 1.2613x; 1.2613x over previous
"""Trainium2 Bass kernel for the 2-layer GAT model (top-10 attention, 4 heads).

Strategy (8 NeuronCores, SPMD):
- Nodes sharded into 8 contiguous ranges of 6250 (dst ranges == GEMM shards).
  Within each core, dst nodes are degree-sorted (host permutation) into 49
  tiles of 128 with a per-tile slot count D[t] shared across cores.
- Two replicated tables per layer: messages xl [NT, 256] f16 (512B rows) and
  attention logits asd [NT, 64] f32 (256B rows; only 8 cols used). The logit
  path stays f32 end-to-end (f16/bf16 logits reorder the top-10 selection
  vs the reference and blow the error budget); messages are f16 (halves
  gather + AllGather traffic vs f32).
- GEMM per shard: message matmuls in f16, att-projection matmuls in f32
  (folded as 8 extra f32 columns); AllGather both tables (27MB + 1.6MB).
  Pad slots point at a poison row in asd (a_s = -1e30) so no explicit pad
  masking is needed; its message row is zero.
- Edge phase: per group of tiles (sum D <= 64) ONE index tile feeds TWO
  dma_gathers (256B logit rows + 512B message rows); alpha = a_s + a_d,
  leaky via fused scalar_tensor_tensor, top-10 threshold via DVE
  max8/match_replace/max8, exp on the scalar engine (f16 out), per-slot f16
  identity-matmul accumulation into PSUM, per-head normalize on the scalar
  engine. b1 folds into GEMM2 (ones-row matmul of b1 @ W2_ext); b2 into the
  MLP bias. Head-mean + 2-layer MLP fused per tile.
"""
import numpy as np

N = 50000
E = 800000
F_IN = 256
H, C = 4, 64
HC = H * C
K_TOP = 10
NEG_SLOPE = 0.2
N_CORES = 8
SH = N // N_CORES            # 6250 real rows per core
TILES = (SH + 127) // 128    # 49
ROWS = TILES * 128           # 6272 padded rows per core
NT = N_CORES * ROWS          # 50176 global table rows
BASE = 32768                 # signed-int16 gather base row
PADROW = NT - 1              # poison row (msg = 0, a_s = -1e30)
HID, OUT_F = 128, 16
GROUP_CAP = 48               # max sum of D over one gather group


def _wrap_idx(vals: np.ndarray) -> np.ndarray:
    """int16 index list -> [128, ceil(len/16)] wrapped+replicated tile."""
    ni = len(vals)
    w = -(-ni // 16)
    arr = np.full(w * 16, PADROW - BASE, np.int16)
    arr[:ni] = vals
    return np.tile(arr.reshape(w, 16).T, (8, 1))


def _prep(x, W1, att_s1, att_d1, W2, att_s2, att_d2, Wl1, Wl2, edge_index,
          b1, b2, bl1):
    """Host preprocessing: sharding, degree-sorted tiles, gather index tables."""
    src = np.asarray(edge_index[0], np.int64)
    dst = np.asarray(edge_index[1], np.int64)

    deg = np.bincount(dst, minlength=N)
    loc = np.empty(N, np.int64)
    node_of = np.full((N_CORES, ROWS), -1, np.int64)
    for c in range(N_CORES):
        nodes = np.arange(c * SH, (c + 1) * SH)
        order = np.argsort(-deg[nodes], kind="stable")
        loc[nodes[order]] = np.arange(SH)
        node_of[c, :SH] = nodes[order]
    rowid = (np.arange(N) // SH) * ROWS + loc  # node -> global table row

    degl = np.zeros((N_CORES, ROWS), np.int64)
    for c in range(N_CORES):
        degl[c, :SH] = deg[node_of[c, :SH]]
    tile_max = degl.reshape(N_CORES, TILES, 128).max(axis=(0, 2))
    D = np.maximum(8, ((tile_max + 3) // 4) * 4).astype(np.int64)
    assert D.max() <= GROUP_CAP, f"degree too high: {D.max()}"

    groups = []
    cur, tot = [], 0
    for t in range(TILES):
        if cur and tot + int(D[t]) > GROUP_CAP:
            groups.append(cur)
            cur, tot = [], 0
        cur.append(t)
        tot += int(D[t])
    groups.append(cur)

    # CSR of edges by (core, local dst row)
    e_loc = (dst // SH) * ROWS + loc[dst]
    order_e = np.argsort(e_loc, kind="stable")
    src_s = src[order_e]
    e_loc_s = e_loc[order_e]
    starts = np.searchsorted(e_loc_s, np.arange(N_CORES * ROWS))
    ends = np.searchsorted(e_loc_s, np.arange(N_CORES * ROWS) + 1)

    PAD16 = np.int16(PADROW - BASE)
    idx_parts = [[] for _ in range(N_CORES)]
    for c in range(N_CORES):
        tile_s16 = []
        for t in range(TILES):
            Dt = int(D[t])
            slot = np.full((128, Dt), PADROW, np.int64)
            for d in range(128):
                r = c * ROWS + t * 128 + d
                s, e = starts[r], ends[r]
                if e > s:
                    slot[d, : e - s] = rowid[src_s[s:e]]
            tile_s16.append((slot - BASE).astype(np.int16).T.reshape(-1))
        for tg in groups:
            vals = np.concatenate([tile_s16[t] for t in tg] + [[PAD16]])
            idx_parts[c].append(_wrap_idx(vals))
    idx = np.stack([np.concatenate(p, axis=1) for p in idx_parts])

    x_shardT = np.zeros((N_CORES, F_IN, ROWS), np.float32)
    xf = np.asarray(x, np.float32)
    for c in range(N_CORES):
        x_shardT[c, :, :SH] = xf[node_of[c, :SH]].T

    def att_fold(WT, att_s, att_d):
        Vs = np.stack([WT[:, h * C: (h + 1) * C] @ np.asarray(att_s)[0, h]
                       for h in range(H)], axis=1)
        Vd = np.stack([WT[:, h * C: (h + 1) * C] @ np.asarray(att_d)[0, h]
                       for h in range(H)], axis=1)
        return np.hstack([Vs, Vd]).astype(np.float32)

    W1T = np.asarray(W1).T.astype(np.float32)
    W2T = np.asarray(W2).T.astype(np.float32)
    V1 = att_fold(W1T, att_s1, att_d1)            # [F_IN, 8] f32
    V2 = att_fold(W2T, att_s2, att_d2)            # [HC, 8] f32
    b1f = np.asarray(b1, np.float32)
    b1w2m = (b1f @ W2T)[None, :]                  # [1, 256]
    b1w2a = (b1f @ V2)[None, :]                   # [1, 8]
    bl1p = (np.asarray(bl1, np.float32)
            + np.asarray(Wl1, np.float32) @ np.asarray(b2, np.float32))

    meta = dict(D=[int(d) for d in D], groups=groups)
    consts = dict(
        W1m=W1T.astype(np.float16), W2m=W2T.astype(np.float16),
        V1=V1, V2=V2,
        b1w2m=b1w2m.astype(np.float16), b1w2a=b1w2a,
        Wl1T=np.asarray(Wl1).T.astype(np.float32).copy(),
        Wl2T=np.asarray(Wl2).T.astype(np.float32).copy(),
        bl1_col=np.ascontiguousarray(bl1p[:, None]),
    )
    per_core = dict(x_shardT=x_shardT, idx=idx)
    return meta, consts, per_core, node_of


def build_gnn(meta, repeat=1, stage=6):
    """stage: 1=gemm1, 2=+ag1, 3=+edge1, 4=+gemm2, 5=+ag2, 6=full."""
    from concourse import bass, bacc, mybir
    import concourse.tile as tile
    from concourse.masks import make_identity

    D = meta["D"]
    groups = meta["groups"]
    WAS = sum(-(-(128 * sum(D[t] for t in tg) + 1) // 16) for tg in groups)

    f32 = mybir.dt.float32
    f16 = mybir.dt.float16
    i16 = mybir.dt.int16
    nc = bacc.Bacc(None, target_bir_lowering=False, num_devices=N_CORES,
                   num_swdge_queues=4)

    xT_in = nc.dram_tensor("x_shardT", [F_IN, ROWS], f32, kind="ExternalInput")
    w1_in = nc.dram_tensor("W1m", [F_IN, HC], f16, kind="ExternalInput")
    w2_in = nc.dram_tensor("W2m", [HC, HC], f16, kind="ExternalInput")
    v1_in = nc.dram_tensor("V1", [F_IN, 8], f32, kind="ExternalInput")
    v2_in = nc.dram_tensor("V2", [HC, 8], f32, kind="ExternalInput")
    bwm_in = nc.dram_tensor("b1w2m", [1, HC], f16, kind="ExternalInput")
    bwa_in = nc.dram_tensor("b1w2a", [1, 8], f32, kind="ExternalInput")
    wl1_in = nc.dram_tensor("Wl1T", [C, HID], f32, kind="ExternalInput")
    wl2_in = nc.dram_tensor("Wl2T", [HID, OUT_F], f32, kind="ExternalInput")
    bl1_in = nc.dram_tensor("bl1_col", [HID, 1], f32, kind="ExternalInput")
    bl2_in = nc.dram_tensor("bl2_rep", [128, OUT_F], f32, kind="ExternalInput")
    ia_in = nc.dram_tensor("idx", [128, WAS], i16, kind="ExternalInput")

    out_dram = nc.dram_tensor("out", [ROWS, OUT_F], f32, kind="ExternalOutput")

    xl_sh = [nc.dram_tensor(f"xl{l}_shard", [ROWS, HC], f16) for l in (1, 2)]
    xl_fu = [nc.dram_tensor(f"xl{l}_full", [NT, HC], f16, addr_space="Shared")
             for l in (1, 2)]
    asd_lo = [nc.dram_tensor(f"asd{l}_local", [ROWS, 8], f32) for l in (1, 2)]
    asd_fu = [nc.dram_tensor(f"asd{l}_full", [NT, 8], f32, addr_space="Shared")
              for l in (1, 2)]
    asd_pad = [nc.dram_tensor(f"asd{l}_pad", [NT, 64], f32) for l in (1, 2)]
    out1_dram = nc.dram_tensor("out1_dram", [ROWS, HC], f32)

    CPY = mybir.ActivationFunctionType.Copy
    EXP = mybir.ActivationFunctionType.Exp
    RELU = mybir.ActivationFunctionType.Relu
    MAXO = mybir.AluOpType.max
    ADD = mybir.AluOpType.add
    MUL = mybir.AluOpType.mult
    SUB = mybir.AluOpType.subtract
    LT = mybir.AluOpType.is_lt
    GT = mybir.AluOpType.is_gt
    X = mybir.AxisListType.X

    with tile.TileContext(nc) as tc:
        with (
            tc.tile_pool(name="const", bufs=1) as cpool,
            tc.tile_pool(name="gemm", bufs=4) as gpool,
            tc.tile_pool(name="gpsum", bufs=2, space="PSUM") as gpsum,
            tc.tile_pool(name="idxp", bufs=3) as ipool,
            tc.tile_pool(name="edge", bufs=3) as epool,
            tc.tile_pool(name="small", bufs=6) as spool,
            tc.tile_pool(name="agg", bufs=2, space="PSUM") as apsum,
            tc.tile_pool(name="mlpp", bufs=1, space="PSUM") as mpsum,
        ):
            # ---- constants ----
            ident = cpool.tile([128, 128], f32)
            make_identity(nc, ident[:])
            identh = cpool.tile([128, 128], f16)
            nc.vector.tensor_copy(identh[:], ident[:])
            ones16 = cpool.tile([1, 128], f16)
            nc.vector.memset(ones16[:], 1.0)
            ones32 = cpool.tile([1, 128], f32)
            nc.vector.memset(ones32[:], 1.0)
            poison = cpool.tile([1, 4], f32)
            nc.vector.memset(poison[:], -1e30)
            w1_sb = cpool.tile([128, 2, HC], f16)
            nc.sync.dma_start(out=w1_sb[:, 0], in_=w1_in[0:128])
            nc.sync.dma_start(out=w1_sb[:, 1], in_=w1_in[128:256])
            w2_sb = cpool.tile([128, 2, HC], f16)
            nc.sync.dma_start(out=w2_sb[:, 0], in_=w2_in[0:128])
            nc.sync.dma_start(out=w2_sb[:, 1], in_=w2_in[128:256])
            v1_sb = cpool.tile([128, 2, 8], f32)
            nc.sync.dma_start(out=v1_sb[:, 0], in_=v1_in[0:128])
            nc.sync.dma_start(out=v1_sb[:, 1], in_=v1_in[128:256])
            v2_sb = cpool.tile([128, 2, 8], f32)
            nc.sync.dma_start(out=v2_sb[:, 0], in_=v2_in[0:128])
            nc.sync.dma_start(out=v2_sb[:, 1], in_=v2_in[128:256])
            bwm_sb = cpool.tile([1, HC], f16)
            nc.sync.dma_start(out=bwm_sb[:], in_=bwm_in[:])
            bwa_sb = cpool.tile([1, 8], f32)
            nc.sync.dma_start(out=bwa_sb[:], in_=bwa_in[:])
            wl1_sb = cpool.tile([C, HID], f32)
            nc.sync.dma_start(out=wl1_sb[:], in_=wl1_in[:])
            wl2_sb = cpool.tile([HID, OUT_F], f32)
            nc.sync.dma_start(out=wl2_sb[:], in_=wl2_in[:])
            bl1_sb = cpool.tile([HID, 1], f32)
            nc.sync.dma_start(out=bl1_sb[:], in_=bl1_in[:])
            bl2_sb = cpool.tile([128, OUT_F], f32)
            nc.sync.dma_start(out=bl2_sb[:], in_=bl2_in[:])
            # all gather-index tables preloaded once (reused by both layers):
            # keeps per-group idx loads out of the in-order SP DMA queue,
            # which otherwise serializes gathers behind compute-dependent
            # output writes
            ia_sb = cpool.tile([128, WAS], i16)
            nc.sync.dma_start(out=ia_sb[:], in_=ia_in[:])

            qrr = [0]

            def gemm_phase(l, w_sb, v_sb):
                li = l - 1
                for t in range(TILES):
                    rows = slice(t * 128, (t + 1) * 128)
                    xT32 = gpool.tile([128, 2, 128], f32, tag="g_T32")
                    xT16 = gpool.tile([128, 2, 128], f16, tag="g_T16")
                    if l == 1:
                        nc.sync.dma_start(out=xT32[:, 0], in_=xT_in[0:128, rows])
                        nc.sync.dma_start(out=xT32[:, 1], in_=xT_in[128:256, rows])
                        for k in range(2):
                            nc.scalar.activation(xT16[:, k], xT32[:, k], CPY)
                    else:
                        o1t = gpool.tile([128, HC], f32, tag="g_in")
                        nc.sync.dma_start(out=o1t[:], in_=out1_dram[rows])
                        for k in range(2):
                            pst = gpsum.tile([128, 128], f32, tag="g_tp")
                            nc.tensor.transpose(
                                pst[:], o1t[:, k * 128: (k + 1) * 128], ident[:])
                            nc.scalar.activation(xT32[:, k], pst[:], CPY)
                            nc.scalar.activation(xT16[:, k], pst[:], CPY)
                    ps = gpsum.tile([128, HC + 8], f32, tag="g_mm")
                    nc.tensor.matmul(ps[:, 0:HC], xT16[:, 0], w_sb[:, 0],
                                     start=True, stop=False)
                    nc.tensor.matmul(ps[:, 0:HC], xT16[:, 1], w_sb[:, 1],
                                     start=False, stop=(l == 1))
                    if l == 2:
                        nc.tensor.matmul(ps[:, 0:HC], ones16[:], bwm_sb[:],
                                         start=False, stop=True)
                    nc.tensor.matmul(ps[:, HC: HC + 8], xT32[:, 0], v_sb[:, 0],
                                     start=True, stop=False)
                    nc.tensor.matmul(ps[:, HC: HC + 8], xT32[:, 1], v_sb[:, 1],
                                     start=False, stop=(l == 1))
                    if l == 2:
                        nc.tensor.matmul(ps[:, HC: HC + 8], ones32[:], bwa_sb[:],
                                         start=False, stop=True)
                    ogm = gpool.tile([128, HC], f16, tag="g_om")
                    nc.scalar.activation(ogm[:], ps[:, 0:HC], CPY)
                    oga = gpool.tile([128, 8], f32, tag="g_oa")
                    nc.vector.tensor_copy(oga[:], ps[:, HC: HC + 8])
                    nc.sync.dma_start(out=xl_sh[li][rows], in_=ogm[:])
                    nc.sync.dma_start(out=asd_lo[li][rows], in_=oga[:])

            def allgather(l):
                li = l - 1
                nc.gpsimd.collective_compute(
                    "AllGather", mybir.AluOpType.bypass,
                    replica_groups=[list(range(N_CORES))],
                    ins=[asd_lo[li].ap().opt()], outs=[asd_fu[li].ap().opt()],
                )
                nc.sync.dma_start(out=asd_pad[li][:, 0:8], in_=asd_fu[li][:])
                nc.sync.dma_start(
                    out=asd_pad[li][PADROW: PADROW + 1, 0:4], in_=poison[:])
                nc.gpsimd.collective_compute(
                    "AllGather", mybir.AluOpType.bypass,
                    replica_groups=[list(range(N_CORES))],
                    ins=[xl_sh[li].ap().opt()], outs=[xl_fu[li].ap().opt()],
                )

            def do_norm(l, rows, ps, inv):
                # normalize + (layer 2) head-mean + MLP; lags one tile so the
                # vector engine is not blocked waiting on this tile's matmuls
                if l == 1:
                    o1 = epool.tile([128, HC], f32, tag="o1")
                    for h in range(H):
                        cs = slice(h * C, (h + 1) * C)
                        nc.scalar.activation(o1[:, cs], ps[:, cs], CPY,
                                             scale=inv[:, h: h + 1])
                    nc.sync.dma_start(out=out1_dram[rows], in_=o1[:])
                else:
                    o2 = spool.tile([128, C], f32, tag="o2")
                    nc.vector.tensor_scalar(
                        out=o2[:], in0=ps[:, 0:C],
                        scalar1=inv[:, 0:1], scalar2=None, op0=MUL)
                    for h in range(1, H):
                        cs = slice(h * C, (h + 1) * C)
                        nc.vector.scalar_tensor_tensor(
                            out=o2[:], in0=ps[:, cs],
                            scalar=inv[:, h: h + 1], in1=o2[:],
                            op0=MUL, op1=ADD)
                    psT = mpsum.tile([C, 128], f32, tag="m_th")
                    nc.tensor.transpose(psT[:], o2[:], ident[:])
                    o2T = spool.tile([C, 128], f32, tag="o2T")
                    nc.scalar.activation(o2T[:], psT[:], CPY)
                    psh = mpsum.tile([HID, 128], f32, tag="m_th")
                    nc.tensor.matmul(psh[:], wl1_sb[:], o2T[:],
                                     start=True, stop=True)
                    rh = spool.tile([HID, 128], f32, tag="rh")
                    nc.scalar.activation(rh[:], psh[:], RELU, bias=bl1_sb[:])
                    pso = mpsum.tile([OUT_F, 128], f32, tag="m_of")
                    nc.tensor.matmul(pso[:], wl2_sb[:], rh[:],
                                     start=True, stop=True)
                    po = spool.tile([OUT_F, 128], f32, tag="po")
                    nc.vector.tensor_copy(po[:], pso[:])
                    psf = mpsum.tile([128, OUT_F], f32, tag="m_of")
                    nc.tensor.transpose(psf[:], po[:], ident[:OUT_F, :OUT_F])
                    of = spool.tile([128, OUT_F], f32, tag="of")
                    nc.vector.tensor_tensor(out=of[:], in0=psf[:],
                                            in1=bl2_sb[:], op=ADD)
                    nc.sync.dma_start(out=out_dram[rows], in_=of[:])

            pending = [None]

            def edge_phase(l):
                li = l - 1
                adr = (asd_lo[li].ap()
                       .rearrange("(t d) c -> d t c", d=128)[:, :, 4:8])
                ad_all = cpool.tile([128, TILES, 4], f32, tag=f"ad{l}")
                nc.sync.dma_start(out=ad_all[:], in_=adr)

                oas = 0
                for tg in groups:
                    SG = sum(D[t] for t in tg)
                    wg = -(-(128 * SG + 1) // 16)
                    ixt = ia_sb[:, oas: oas + wg]
                    oas += wg
                    asg = epool.tile([128, SG + 1, 64], f32, tag="asg")
                    nc.gpsimd.dma_gather(
                        out_ap=asg[:], in_ap=asd_pad[li][BASE:, :],
                        idxs_ap=ixt, num_idxs=128 * SG + 1,
                        num_idxs_reg=128 * SG + 1, elem_size=64,
                        single_packet=False, queue_num=qrr[0] % 4,
                    )
                    qrr[0] += 1
                    xg = epool.tile([128, SG + 1, HC], f16, tag="xg")
                    nc.gpsimd.dma_gather(
                        out_ap=xg[:], in_ap=xl_fu[li][BASE:, :], idxs_ap=ixt,
                        num_idxs=128 * SG + 1, num_idxs_reg=128 * SG + 1,
                        elem_size=HC, single_packet=False,
                        queue_num=qrr[0] % 4,
                    )
                    qrr[0] += 1
                    off = 0
                    for t in tg:
                        Dt = D[t]
                        rows = slice(t * 128, (t + 1) * 128)
                        msl = xg[:, off: off + Dt]
                        alpha = spool.tile([128, H, Dt], f32, tag="alpha")
                        nc.vector.tensor_tensor(
                            out=alpha[:],
                            in0=asg[:, off: off + Dt, 0:4].transpose([0, 2, 1]),
                            in1=ad_all[:, t].unsqueeze(2)
                                .broadcast_to([128, H, Dt]),
                            op=ADD,
                        )
                        nc.vector.scalar_tensor_tensor(
                            out=alpha[:], in0=alpha[:], scalar=NEG_SLOPE,
                            in1=alpha[:], op0=MUL, op1=MAXO,
                        )
                        m8a = spool.tile([128, H, 8], f32, tag="m8a")
                        exm = spool.tile([128, H, Dt], f32, tag="exm")
                        if K_TOP < Dt <= 17:
                            # rank-10-largest == ascending rank Dt-10 (<8):
                            # ONE max8 on -alpha per head replaces
                            # max8 + match_replace + max8
                            nal = spool.tile([128, H, Dt], f32, tag="nal")
                            nc.vector.tensor_scalar_mul(nal[:], alpha[:], -1.0)
                            nc.vector.reduce_max(out=m8a[:, :, 0], in_=alpha[:],
                                                 axis=X)
                            m8n = spool.tile([128, H, 8], f32, tag="m8n")
                            for h in range(H):
                                nc.vector.max(out=m8n[:, h], in_=nal[:, h])
                            nc.vector.tensor_tensor(
                                out=exm[:], in0=alpha[:],
                                in1=m8a[:, :, 0:1].broadcast_to([128, H, Dt]),
                                op=SUB)
                            mlt = spool.tile([128, H, Dt], f32, tag="mlt")
                            ks = Dt - K_TOP
                            nc.vector.tensor_tensor(
                                out=mlt[:], in0=nal[:],
                                in1=m8n[:, :, ks: ks + 1]
                                    .broadcast_to([128, H, Dt]),
                                op=GT)
                            nc.vector.scalar_tensor_tensor(
                                out=exm[:], in0=mlt[:], scalar=-1e30,
                                in1=exm[:], op0=MUL, op1=ADD)
                        elif Dt > K_TOP:
                            m8b = spool.tile([128, H, 8], f32, tag="m8b")
                            wk = spool.tile([128, Dt], f32, tag="wk")
                            for h in range(H):
                                nc.vector.max(out=m8a[:, h], in_=alpha[:, h])
                                nc.vector.match_replace(
                                    out=wk[:], in_to_replace=m8a[:, h],
                                    in_values=alpha[:, h], imm_value=-3e30)
                                nc.vector.max(out=m8b[:, h], in_=wk[:])
                            nc.vector.tensor_tensor(
                                out=exm[:], in0=alpha[:],
                                in1=m8a[:, :, 0:1].broadcast_to([128, H, Dt]),
                                op=SUB)
                            mlt = spool.tile([128, H, Dt], f32, tag="mlt")
                            nc.vector.tensor_tensor(
                                out=mlt[:], in0=alpha[:],
                                in1=m8b[:, :, 1:2].broadcast_to([128, H, Dt]),
                                op=LT)
                            nc.vector.scalar_tensor_tensor(
                                out=exm[:], in0=mlt[:], scalar=-1e30,
                                in1=exm[:], op0=MUL, op1=ADD)
                        else:
                            nc.vector.reduce_max(out=m8a[:, :, 0], in_=alpha[:],
                                                 axis=X)
                            nc.vector.tensor_scalar_max(
                                m8a[:, :, 0], m8a[:, :, 0], -1e29)
                            nc.vector.tensor_tensor(
                                out=exm[:], in0=alpha[:],
                                in1=m8a[:, :, 0:1].broadcast_to([128, H, Dt]),
                                op=SUB)
                        exb = spool.tile([128, H, Dt], f16, tag="exb")
                        nc.scalar.activation(exb[:], exm[:], EXP)
                        den = spool.tile([128, H], f32, tag="den")
                        nc.vector.reduce_sum(out=den[:], in_=exb[:], axis=X)
                        inv = spool.tile([128, H], f32, tag="inv")
                        if l == 1:
                            nc.vector.tensor_scalar_max(den[:], den[:], 1e-20)
                        else:
                            nc.vector.tensor_scalar(
                                out=den[:], in0=den[:], scalar1=1e-20,
                                scalar2=float(H), op0=MAXO, op1=MUL)
                        nc.vector.reciprocal(inv[:], den[:])
                        nc.vector.tensor_tensor(
                            out=msl[:].rearrange("p j (h c) -> p j h c", h=H),
                            in0=msl[:].rearrange("p j (h c) -> p j h c", h=H),
                            in1=exb[:].transpose([0, 2, 1]).unsqueeze(3)
                                .broadcast_to([128, Dt, H, C]),
                            op=MUL,
                        )
                        ps = apsum.tile([128, HC], f32, tag="agg")
                        for j in range(Dt):
                            nc.tensor.matmul(
                                ps[:], identh[:], xg[:, off + j],
                                start=(j == 0), stop=(j == Dt - 1))
                        prev = pending[0]
                        pending[0] = (rows, ps, inv)
                        if prev is not None:
                            do_norm(l, *prev)
                        off += Dt
                if pending[0] is not None:
                    do_norm(l, *pending[0])
                    pending[0] = None

            def edge_gather_only(l):
                # stage-7 probe: run the full gather stream of one edge phase
                # with minimal consumption, to isolate DMA cost
                li = l - 1
                cons = cpool.tile([128, 8], f32, tag="cons")
                nc.vector.memset(cons[:], 0.0)
                oas = 0
                for tg in groups:
                    SG = sum(D[t] for t in tg)
                    wg = -(-(128 * SG + 1) // 16)
                    ixt = ia_sb[:, oas: oas + wg]
                    oas += wg
                    asg = epool.tile([128, SG + 1, 64], f32, tag="asg")
                    nc.gpsimd.dma_gather(
                        out_ap=asg[:], in_ap=asd_pad[li][BASE:, :],
                        idxs_ap=ixt, num_idxs=128 * SG + 1,
                        num_idxs_reg=128 * SG + 1, elem_size=64,
                        single_packet=False, queue_num=qrr[0] % 4,
                    )
                    qrr[0] += 1
                    xg = epool.tile([128, SG + 1, HC], f16, tag="xg")
                    nc.gpsimd.dma_gather(
                        out_ap=xg[:], in_ap=xl_fu[li][BASE:, :], idxs_ap=ixt,
                        num_idxs=128 * SG + 1, num_idxs_reg=128 * SG + 1,
                        elem_size=HC, single_packet=False,
                        queue_num=qrr[0] % 4,
                    )
                    qrr[0] += 1
                    xc = spool.tile([128, 8], f32, tag="xc")
                    nc.vector.tensor_copy(xc[:], xg[:, 0, 0:8])
                    nc.vector.tensor_tensor(out=cons[:], in0=cons[:],
                                            in1=xc[:], op=ADD)
                    nc.vector.tensor_tensor(out=cons[:], in0=cons[:],
                                            in1=asg[:, 0, 0:8], op=ADD)
                nc.sync.dma_start(out=out_dram[0:128, 0:8], in_=cons[:])

            def debug_touch(src_dram, is16):
                # read back a strip so partial stages aren't dead code
                for t in range(TILES):
                    rows = slice(t * 128, (t + 1) * 128)
                    dt_ = spool.tile([128, OUT_F], f16 if is16 else f32,
                                     tag="dbg")
                    nc.sync.dma_start(out=dt_[:], in_=src_dram[rows, 0:OUT_F])
                    df = spool.tile([128, OUT_F], f32, tag="dbgf")
                    nc.vector.tensor_copy(df[:], dt_[:])
                    nc.sync.dma_start(out=out_dram[rows], in_=df[:])

            for _rep in range(repeat):
                if stage == 7:
                    gemm_phase(1, w1_sb, v1_sb)
                    allgather(1)
                    edge_gather_only(1)
                    continue
                gemm_phase(1, w1_sb, v1_sb)
                if stage >= 2:
                    allgather(1)
                if stage >= 3:
                    edge_phase(1)
                if stage >= 4:
                    gemm_phase(2, w2_sb, v2_sb)
                if stage >= 5:
                    allgather(2)
                if stage >= 6:
                    edge_phase(2)
            if stage < 6:
                if stage == 1:
                    debug_touch(xl_sh[0], True)
                elif stage == 2:
                    debug_touch(xl_fu[0], True)
                    debug_touch(asd_pad[0], False)
                elif stage == 3:
                    debug_touch(out1_dram, False)
                elif stage == 4:
                    debug_touch(xl_sh[1], True)
                elif stage == 5:
                    debug_touch(xl_fu[1], True)
                    debug_touch(asd_pad[1], False)

    nc.compile()
    return nc


def _make_in_maps(consts, per_core, bl2):
    bl2 = np.asarray(bl2, np.float32)
    shared = dict(
        W1m=consts["W1m"], W2m=consts["W2m"], V1=consts["V1"], V2=consts["V2"],
        b1w2m=consts["b1w2m"], b1w2a=consts["b1w2a"],
        Wl1T=consts["Wl1T"], Wl2T=consts["Wl2T"],
        bl1_col=consts["bl1_col"],
        bl2_rep=np.tile(bl2[None, :], (128, 1)),
    )
    return [
        dict(
            shared,
            x_shardT=np.ascontiguousarray(per_core["x_shardT"][c]),
            idx=np.ascontiguousarray(per_core["idx"][c]),
        )
        for c in range(N_CORES)
    ]


def _assemble(results, node_of):
    out = np.empty((N, OUT_F), np.float32)
    for c in range(N_CORES):
        out[node_of[c, :SH]] = results[c]["out"][:SH]
    return out


def kernel(x, W1, att_s1, att_d1, b1, W2, att_s2, att_d2, b2,
           Wl1, bl1, Wl2, bl2, edge_index):
    from concourse.bass_utils import run_bass_kernel_spmd

    meta, consts, per_core, node_of = _prep(
        x, W1, att_s1, att_d1, W2, att_s2, att_d2, Wl1, Wl2, edge_index,
        b1, b2, bl1,
    )
    nc = build_gnn(meta)
    in_maps = _make_in_maps(consts, per_core, bl2)
    res = run_bass_kernel_spmd(nc, in_maps, core_ids=list(range(N_CORES)))
    return _assemble(res.results, node_of)
